# revision 32
# baseline (speedup 1.0000x reference)
"""Llama3 attention prefill kernel for 8 Trainium2 NeuronCores.

Sharding: tensor-parallel over heads. Core c owns Q heads 4c..4c+3 and KV
head c (GQA group), plus the matching wqkv columns / wo rows. Each core
computes a partial output y_c = attn_c @ wo_c; the host sums the partials.

Per-core pipeline (all inside one TileContext):
  1. qkv = x @ w_shard   (fp16 matmuls, xT tiles produced by PE transpose)
  2. RoPE on q/k in [S, head*64*2] layout (DVE), transpose q/k to [HD, S]
  3. causal flash attention per (head, q-tile): scores -> exp (+row sums)
     -> normalize -> PE-transpose P -> P^T @ v accumulation (out^T layout)
  4. y^T = wo_shard^T @ out^T  (float32r matmuls), DMA y^T back
"""

import os
import sys

for _p in ("/opt/trn_rl_repo", "/root/.axon_site/_ro/trn_rl_repo"):
    if os.path.isdir(_p) and _p not in sys.path:
        sys.path.insert(0, _p)

import numpy as np

S = 2048
H = 4096
HD = 128
NQ = 4            # q heads per core
MQKV = 768        # per-core qkv columns: 512 q + 128 k + 128 v
N_CORES = 8
SCALE = 1.0 / float(np.sqrt(HD))
MASK_VAL = -1e9

_CACHE = {}
LAST_RESULTS = None


def _build():
    import concourse.bass as bass
    import concourse.tile as tile
    from concourse import bacc, mybir
    from concourse.masks import make_causal_mask, make_identity

    f32 = mybir.dt.float32
    f32r = mybir.dt.float32r
    f16 = mybir.dt.float16
    Exp = mybir.ActivationFunctionType.Exp

    nc = bacc.Bacc("TRN2", target_bir_lowering=False, debug=False)

    x_ap = nc.dram_tensor("x", [S, H], f32, kind="ExternalInput").ap()
    w_ap = nc.dram_tensor("w", [H, MQKV], f32, kind="ExternalInput").ap()
    wo_ap = nc.dram_tensor("wo", [NQ * HD, H], f32, kind="ExternalInput").ap()
    cs_ap = nc.dram_tensor("cs5", [S, 320], f32, kind="ExternalInput").ap()
    sn_ap = nc.dram_tensor("sn5", [S, 320], f32, kind="ExternalInput").ap()
    yT_ap = nc.dram_tensor("yT", [H, S], f32, kind="ExternalOutput").ap()

    KT = S // 128    # 16 m-tiles over S
    KC = H // 128    # 32 contraction chunks for qkv

    with tile.TileContext(nc) as tc:
        from contextlib import ExitStack

        with ExitStack() as ctx:
            const = ctx.enter_context(tc.tile_pool(name="const", bufs=1))
            ident = const.tile([128, 128], f16)
            make_identity(nc, ident[:])
            cmask = const.tile([128, 128], f32)
            make_causal_mask(nc, cmask[:], mask_val=MASK_VAL)

            # resident tensors
            res = ctx.enter_context(tc.tile_pool(name="res", bufs=1))
            qkT_sb = res.tile([128, 5, KT, 128], f16, name="qkT_sb")
            v_sb = res.tile([128, KT, 128], f16, name="v_sb")
            outT_sb = res.tile([128, NQ, S], f16, name="outT_sb")
            w_pool_cm = tc.tile_pool(name="w_pool", bufs=1, side="right")
            w_pool = w_pool_cm.__enter__()
            w_sb = w_pool.tile([128, KC, MQKV], f16, name="w_sb")

            # streaming pools
            wst = ctx.enter_context(tc.tile_pool(name="wst", bufs=3))
            qsbp = ctx.enter_context(tc.tile_pool(name="qsbp", bufs=2))
            xp = ctx.enter_context(tc.tile_pool(name="xp", bufs=6))
            xfp = ctx.enter_context(tc.tile_pool(name="xfp", bufs=10))
            xtp = ctx.enter_context(tc.tile_pool(name="xtp", bufs=12))
            csp = ctx.enter_context(tc.tile_pool(name="csp", bufs=2))
            rtp = ctx.enter_context(tc.tile_pool(name="rtp", bufs=2))
            rotp = ctx.enter_context(tc.tile_pool(name="rotp", bufs=2))
            pp = ctx.enter_context(tc.tile_pool(name="pp", bufs=5))
            ptp = ctx.enter_context(tc.tile_pool(name="ptp", bufs=3))
            lp = ctx.enter_context(tc.tile_pool(name="lp", bufs=3))
            yp = ctx.enter_context(tc.tile_pool(name="yp", bufs=3))

            # PSUM pools: 4 + 2 + 1 + 1 = 8 banks
            ps_big = ctx.enter_context(
                tc.tile_pool(name="ps_big", bufs=4, space="PSUM"))
            ps_qkv = ctx.enter_context(
                tc.tile_pool(name="ps_qkv", bufs=1, space="PSUM"))
            ps_tr = ctx.enter_context(
                tc.tile_pool(name="ps_tr", bufs=1, space="PSUM"))
            ps_pv = ctx.enter_context(
                tc.tile_pool(name="ps_pv", bufs=1, space="PSUM"))

            # ---- phase 1: qkv projection + rope + q/k transpose ----
            # software-pipelined: x for tile t+1 is loaded/cast/transposed
            # while tile t's matmuls run, so PE never waits at tile bounds.
            def x_load(t):
                casts = []
                for c8 in range(KC // 4):
                    xt = xp.tile([128, 512], f32, tag="xt", name=f"xt{t}_{c8}")
                    nc.sync.dma_start(
                        out=xt[:],
                        in_=x_ap[t * 128:(t + 1) * 128, c8 * 512:(c8 + 1) * 512])
                    if t == 0:
                        for kcw in range(c8 * 4, c8 * 4 + 4):
                            wt = wst.tile([128, MQKV], f32, tag="wt",
                                          name=f"wt{kcw}")
                            nc.sync.dma_start(
                                out=wt[:],
                                in_=w_ap[kcw * 128:(kcw + 1) * 128, :])
                            nc.vector.tensor_copy(out=w_sb[:, kcw, :],
                                                  in_=wt[:])
                    xf = xfp.tile([128, 512], f16, tag="xf", name=f"xf{t}_{c8}")
                    nc.vector.tensor_copy(out=xf[:], in_=xt[:])
                    casts.append(xf)
                return casts

            def x_transpose(t, casts):
                outs = []
                for c8 in range(KC // 4):
                    xf = casts[c8]
                    tr = ps_tr.tile([128, 512], f16, tag="tr",
                                    name=f"xtr{t}_{c8}")
                    for c4 in range(4):
                        nc.tensor.transpose(
                            tr[:, c4 * 128:(c4 + 1) * 128],
                            xf[:, c4 * 128:(c4 + 1) * 128], ident[:])
                    xT = xtp.tile([128, 512], f16, tag="xT",
                                  name=f"xT{t}_{c8}")
                    nc.vector.tensor_copy(out=xT[:], in_=tr[:])
                    outs.append(xT)
                return outs

            xT_cur = x_transpose(0, x_load(0))
            for t in range(KT):
                if t + 1 < KT:
                    casts_next = x_load(t + 1)
                qkv_ps = ps_qkv.tile([128, MQKV], f32, tag="qkv")
                for kc in range(KC):
                    lhsT = xT_cur[kc // 4][:, (kc % 4) * 128:(kc % 4 + 1) * 128]
                    nc.tensor.matmul(
                        qkv_ps[:, 0:512], lhsT=lhsT, rhs=w_sb[:, kc, 0:512],
                        start=(kc == 0), stop=(kc == KC - 1))
                    nc.tensor.matmul(
                        qkv_ps[:, 512:768], lhsT=lhsT, rhs=w_sb[:, kc, 512:768],
                        start=(kc == 0), stop=(kc == KC - 1))
                if t + 1 < KT:
                    xT_cur = x_transpose(t + 1, casts_next)

                # evict full qkv psum to SBUF fast (frees PSUM for next tile)
                qkv_sb = qsbp.tile([128, MQKV], f32, tag="qkv_sb")
                nc.scalar.copy(out=qkv_sb[:], in_=qkv_ps[:])
                # v eviction (no rope)
                nc.scalar.copy(out=v_sb[:, t, :], in_=qkv_sb[:, 640:768])

                # rope on q (4 heads) + k (1 head), pairs interleaved along free
                cs_t = csp.tile([128, 320], f32, tag="cs")
                nc.sync.dma_start(out=cs_t[:], in_=cs_ap[t * 128:(t + 1) * 128, :])
                sn_t = csp.tile([128, 320], f32, tag="sn")
                nc.sync.dma_start(out=sn_t[:], in_=sn_ap[t * 128:(t + 1) * 128, :])

                qk = qkv_sb[:, 0:640].rearrange("p (n two) -> p n two", two=2)
                qe = qk[:, :, 0]
                qo = qk[:, :, 1]
                rot = rotp.tile([128, 640], f16, tag="rot")
                rv = rot[:].rearrange("p (n two) -> p n two", two=2)
                t1 = rtp.tile([128, 320], f32, tag="t1")
                t2 = rtp.tile([128, 320], f32, tag="t2")
                nc.vector.tensor_mul(t1[:], qe, cs_t[:])
                nc.vector.tensor_mul(t2[:], qo, sn_t[:])
                nc.vector.scalar_tensor_tensor(
                    rv[:, :, 0], t2[:], -1.0, t1[:],
                    op0=mybir.AluOpType.mult, op1=mybir.AluOpType.add)
                nc.vector.tensor_mul(t1[:], qo, cs_t[:])
                nc.vector.tensor_mul(t2[:], qe, sn_t[:])
                nc.vector.tensor_add(rv[:, :, 1], t1[:], t2[:])

                # transpose rope'd q/k into [HD, head, t, 128] resident layout
                tr2 = ps_tr.tile([128, 512], f16, tag="tr")
                for h in range(4):
                    nc.tensor.transpose(
                        tr2[:, h * 128:(h + 1) * 128],
                        rot[:, h * 128:(h + 1) * 128], ident[:])
                nc.vector.tensor_copy(
                    out=qkT_sb[:, 0:4, t, :],
                    in_=tr2[:].rearrange("p (h s) -> p h s", h=4))
                tr3 = ps_tr.tile([128, 512], f16, tag="tr")
                nc.tensor.transpose(tr3[:, 0:128], rot[:, 512:640], ident[:])
                nc.vector.tensor_copy(out=qkT_sb[:, 4, t, :], in_=tr3[:, 0:128])

            # ---- phase 1 done: release w_sb space, load wo shard there
            w_pool_cm.__exit__(None, None, None)
            wo_pool = ctx.enter_context(tc.tile_pool(name="wo_pool", bufs=1, side="right"))
            wo_sb = wo_pool.tile([128, NQ, H], f16, name="wo_sb")
            wol = ctx.enter_context(tc.tile_pool(name="wol", bufs=2))
            for kc in range(NQ):
                for hh in range(4):
                    wt3 = wol.tile([128, 1024], f32, tag="wt3")
                    nc.sync.dma_start(
                        out=wt3[:],
                        in_=wo_ap[kc * 128:(kc + 1) * 128,
                                  hh * 1024:(hh + 1) * 1024])
                    nc.vector.tensor_copy(
                        out=wo_sb[:, kc, hh * 1024:(hh + 1) * 1024], in_=wt3[:])

            # ---- phase 2+3: causal flash attention with interleaved output
            # projection (y chunk q4 emitted once q-tiles 4*q4..4*q4+3 done)
            kT_flat = qkT_sb[:, 4, :, :].rearrange("p a b -> p (a b)")
            for i in range(KT):
                L = (i + 1) * 128
                nch = (L + 511) // 512
                Ps = []
                # wave 1: scores + exp + normalization chain for all heads
                for h in range(NQ):
                    P = pp.tile([128, S], f16, tag="P", name=f"P{i}_{h}")
                    lacc = lp.tile([128, 4], f32, tag="l")
                    for ch in range(nch):
                        c0 = ch * 512
                        c1 = min(L, c0 + 512)
                        sps = ps_big.tile([128, 512], f32, tag="big")
                        nc.tensor.matmul(
                            sps[:, 0:c1 - c0],
                            lhsT=qkT_sb[:, h, i, :],
                            rhs=kT_flat[:, c0:c1],
                            start=True, stop=True)
                        if c1 == L:
                            # diagonal block: additive causal mask
                            nc.vector.tensor_add(
                                sps[:, L - 128 - c0:L - c0],
                                sps[:, L - 128 - c0:L - c0], cmask[:])
                        nc.scalar.activation(
                            P[:, c0:c1], sps[:, 0:c1 - c0], Exp,
                            scale=SCALE,
                            accum_out=lacc[:, ch:ch + 1])
                    lsum = lp.tile([128, 1], f32, tag="ls")
                    if nch > 1:
                        nc.vector.tensor_reduce(
                            lsum[:], lacc[:, 0:nch],
                            axis=mybir.AxisListType.X, op=mybir.AluOpType.add)
                    else:
                        nc.vector.tensor_copy(out=lsum[:], in_=lacc[:, 0:1])
                    rinv = lp.tile([128, 1], f32, tag="r")
                    nc.vector.reciprocal(rinv[:], lsum[:])
                    nc.vector.tensor_scalar_mul(P[:, 0:L], P[:, 0:L], rinv[:])
                    Ps.append(P)
                # wave 2: transpose P and accumulate P^T @ v per head
                for h in range(NQ):
                    Pn = Ps[h]
                    PT = ptp.tile([128, S], f16, tag="PT", name=f"PT{i}_{h}")
                    for j4 in range(0, i + 1, 4):
                        jn = min(i + 1, j4 + 4)
                        trp = ps_tr.tile([128, 512], f16, tag="tr")
                        for jj in range(j4, jn):
                            nc.tensor.transpose(
                                trp[:, (jj - j4) * 128:(jj - j4 + 1) * 128],
                                Pn[:, jj * 128:(jj + 1) * 128], ident[:])
                        if (j4 // 4) % 2 == 0:
                            nc.scalar.copy(
                                out=PT[:, j4 * 128:jn * 128],
                                in_=trp[:, 0:(jn - j4) * 128])
                        else:
                            nc.vector.tensor_copy(
                                out=PT[:, j4 * 128:jn * 128],
                                in_=trp[:, 0:(jn - j4) * 128])

                    ov = ps_pv.tile([128, 128], f32, tag="pv")
                    for j in range(i + 1):
                        nc.tensor.matmul(
                            ov[:], lhsT=v_sb[:, j, :],
                            rhs=PT[:, j * 128:(j + 1) * 128],
                            start=(j == 0), stop=(j == i))
                    nc.vector.tensor_copy(
                        out=outT_sb[:, h, i * 128:(i + 1) * 128], in_=ov[:])

                yq4s = []
                if i % 4 == 3 and i >= 7:
                    yq4s = [i // 4 - 1]
                if i == KT - 1:
                    yq4s = [KT // 4 - 2, KT // 4 - 1]
                for q4 in yq4s:
                    for ym in range(H // 128):
                        yps = ps_big.tile([128, 512], f32, tag="big")
                        for kc in range(NQ):
                            nc.tensor.matmul(
                                yps[:],
                                lhsT=wo_sb[:, kc, ym * 128:(ym + 1) * 128],
                                rhs=outT_sb[:, kc, q4 * 512:(q4 + 1) * 512],
                                start=(kc == 0), stop=(kc == NQ - 1))
                        yev = yp.tile([128, 512], f32, tag="yev")
                        if ym % 2 == 0:
                            nc.scalar.copy(out=yev[:], in_=yps[:])
                        else:
                            nc.vector.tensor_copy(out=yev[:], in_=yps[:])
                        nc.sync.dma_start(
                            out=yT_ap[ym * 128:(ym + 1) * 128,
                                      q4 * 512:(q4 + 1) * 512],
                            in_=yev[:])

    nc.compile()
    return nc


def _get_nc():
    if "nc" not in _CACHE:
        _CACHE["nc"] = _build()
    return _CACHE["nc"]


def kernel(x, last_pos, mask, rope_cache, wqkv, wo):
    global LAST_RESULTS
    from concourse.bass_utils import run_bass_kernel_spmd

    nc = _get_nc()

    x2 = np.ascontiguousarray(np.asarray(x, np.float32).reshape(S, H))
    rc = np.asarray(rope_cache, np.float32)          # [S, 64, 2]
    cos = rc[:, :, 0]                                # [S, 64]
    sin = rc[:, :, 1]
    # per-pair factors, tiled for 5 rope'd heads (4 q + 1 k): [S, 320]
    cs5 = np.ascontiguousarray(np.tile(cos, (1, 5)))
    sn5 = np.ascontiguousarray(np.tile(sin, (1, 5)))
    wq = np.asarray(wqkv, np.float32)
    wo_f = np.asarray(wo, np.float32)

    in_maps = []
    for c in range(N_CORES):
        wcat = np.concatenate(
            [wq[:, c * 512:(c + 1) * 512],
             wq[:, H + c * 128:H + (c + 1) * 128],
             wq[:, H + 1024 + c * 128:H + 1024 + (c + 1) * 128]],
            axis=1)
        in_maps.append({
            "x": x2,
            "w": np.ascontiguousarray(wcat),
            "wo": np.ascontiguousarray(wo_f[c * 512:(c + 1) * 512, :]),
            "cs5": cs5,
            "sn5": sn5,
        })

    res = run_bass_kernel_spmd(nc, in_maps, list(range(N_CORES)))
    LAST_RESULTS = res
    if res.exec_time_ns is not None:
        print(f"HW exec time: {res.exec_time_ns} ns")
    yT = res.results[0]["yT"].astype(np.float64)
    for c in range(1, N_CORES):
        yT = yT + res.results[c]["yT"]
    return np.ascontiguousarray(yT.T).reshape(1, S, H).astype(np.float32)


# revision 38
# speedup vs baseline: 1.0043x; 1.0043x over previous
"""Llama3 attention prefill kernel for 8 Trainium2 NeuronCores.

Sharding: tensor-parallel over heads. Core c owns Q heads 4c..4c+3 and KV
head c (GQA group), plus the matching wqkv columns / wo rows. Each core
computes a partial output y_c = attn_c @ wo_c; the host sums the partials.

Per-core pipeline (all inside one TileContext):
  1. qkv = x @ w_shard   (fp16 matmuls, xT tiles produced by PE transpose)
  2. RoPE on q/k in [S, head*64*2] layout (DVE), transpose q/k to [HD, S]
  3. causal flash attention per (head, q-tile): scores -> exp (+row sums)
     -> normalize -> PE-transpose P -> P^T @ v accumulation (out^T layout)
  4. y^T = wo_shard^T @ out^T  (float32r matmuls), DMA y^T back
"""

import os
import sys

for _p in ("/opt/trn_rl_repo", "/root/.axon_site/_ro/trn_rl_repo"):
    if os.path.isdir(_p) and _p not in sys.path:
        sys.path.insert(0, _p)

import numpy as np

S = 2048
H = 4096
HD = 128
NQ = 4            # q heads per core
MQKV = 768        # per-core qkv columns: 512 q + 128 k + 128 v
N_CORES = 8
SCALE = 1.0 / float(np.sqrt(HD))
MASK_VAL = -1e9

_CACHE = {}
LAST_RESULTS = None


def _build():
    import concourse.tile as tile
    from concourse import bacc, mybir
    from concourse.masks import make_causal_mask, make_identity

    f32 = mybir.dt.float32
    f16 = mybir.dt.float16
    Exp = mybir.ActivationFunctionType.Exp

    nc = bacc.Bacc("TRN2", target_bir_lowering=False, debug=False)

    x_ap = nc.dram_tensor("x", [S, H], f32, kind="ExternalInput").ap()
    w_ap = nc.dram_tensor("w", [H, MQKV], f32, kind="ExternalInput").ap()
    wo_ap = nc.dram_tensor("wo", [NQ * HD, H], f32, kind="ExternalInput").ap()
    cs_ap = nc.dram_tensor("cs5", [S, 320], f32, kind="ExternalInput").ap()
    sn_ap = nc.dram_tensor("sn5", [S, 320], f32, kind="ExternalInput").ap()
    yT_ap = nc.dram_tensor("yT", [H, S], f32, kind="ExternalOutput").ap()

    KT = S // 128    # 16 m-tiles over S
    KC = H // 128    # 32 contraction chunks for qkv

    with tile.TileContext(nc) as tc:
        from contextlib import ExitStack

        with ExitStack() as ctx:
            const = ctx.enter_context(tc.tile_pool(name="const", bufs=1))
            ident = const.tile([128, 128], f16)
            make_identity(nc, ident[:])
            cmask = const.tile([128, 128], f32)
            make_causal_mask(nc, cmask[:], mask_val=MASK_VAL)

            # resident tensors
            res = ctx.enter_context(tc.tile_pool(name="res", bufs=1))
            qkT_sb = res.tile([128, 5, KT, 128], f16, name="qkT_sb")
            v_sb = res.tile([128, KT, 128], f16, name="v_sb")
            outT_sb = res.tile([128, NQ, S], f16, name="outT_sb")
            w_pool_cm = tc.tile_pool(name="w_pool", bufs=1, side="right")
            w_pool = w_pool_cm.__enter__()
            w_sb = w_pool.tile([128, KC, MQKV], f16, name="w_sb")

            # streaming pools
            wst = ctx.enter_context(tc.tile_pool(name="wst", bufs=3))
            qsbp = ctx.enter_context(tc.tile_pool(name="qsbp", bufs=2))
            xp = ctx.enter_context(tc.tile_pool(name="xp", bufs=6))
            xfp = ctx.enter_context(tc.tile_pool(name="xfp", bufs=10))
            xtp = ctx.enter_context(tc.tile_pool(name="xtp", bufs=12))
            csp = ctx.enter_context(tc.tile_pool(name="csp", bufs=2))
            rtp = ctx.enter_context(tc.tile_pool(name="rtp", bufs=2))
            rotp = ctx.enter_context(tc.tile_pool(name="rotp", bufs=2))
            pp = ctx.enter_context(tc.tile_pool(name="pp", bufs=5))
            ptp = ctx.enter_context(tc.tile_pool(name="ptp", bufs=3))
            lp = ctx.enter_context(tc.tile_pool(name="lp", bufs=3))
            yp = ctx.enter_context(tc.tile_pool(name="yp", bufs=3))

            # PSUM pools: 4 + 2 + 1 + 1 = 8 banks
            ps_big = ctx.enter_context(
                tc.tile_pool(name="ps_big", bufs=4, space="PSUM"))
            ps_qkv = ctx.enter_context(
                tc.tile_pool(name="ps_qkv", bufs=1, space="PSUM"))
            ps_tr = ctx.enter_context(
                tc.tile_pool(name="ps_tr", bufs=1, space="PSUM"))
            ps_pv = ctx.enter_context(
                tc.tile_pool(name="ps_pv", bufs=1, space="PSUM"))

            # ---- phase 1: qkv projection + rope + q/k transpose ----
            # software-pipelined: x for tile t+1 is loaded/cast/transposed
            # while tile t's matmuls run, so PE never waits at tile bounds.
            def x_load(t):
                casts = []
                for c8 in range(KC // 4):
                    xt = xp.tile([128, 512], f32, tag="xt", name=f"xt{t}_{c8}")
                    nc.sync.dma_start(
                        out=xt[:],
                        in_=x_ap[t * 128:(t + 1) * 128, c8 * 512:(c8 + 1) * 512])
                    if t == 0:
                        for kcw in range(c8 * 4, c8 * 4 + 4):
                            wt = wst.tile([128, MQKV], f32, tag="wt",
                                          name=f"wt{kcw}")
                            nc.sync.dma_start(
                                out=wt[:],
                                in_=w_ap[kcw * 128:(kcw + 1) * 128, :])
                            nc.vector.tensor_copy(out=w_sb[:, kcw, :],
                                                  in_=wt[:])
                    xf = xfp.tile([128, 512], f16, tag="xf", name=f"xf{t}_{c8}")
                    nc.vector.tensor_copy(out=xf[:], in_=xt[:])
                    casts.append(xf)
                return casts

            def x_transpose(t, casts):
                outs = []
                for c8 in range(KC // 4):
                    xf = casts[c8]
                    tr = ps_tr.tile([128, 512], f16, tag="tr",
                                    name=f"xtr{t}_{c8}")
                    for c4 in range(4):
                        nc.tensor.transpose(
                            tr[:, c4 * 128:(c4 + 1) * 128],
                            xf[:, c4 * 128:(c4 + 1) * 128], ident[:])
                    xT = xtp.tile([128, 512], f16, tag="xT",
                                  name=f"xT{t}_{c8}")
                    nc.vector.tensor_copy(out=xT[:], in_=tr[:])
                    outs.append(xT)
                return outs

            xT_cur = x_transpose(0, x_load(0))
            for t in range(KT):
                if t + 1 < KT:
                    casts_next = x_load(t + 1)
                qkv_ps = ps_qkv.tile([128, MQKV], f32, tag="qkv")
                for kc in range(KC):
                    lhsT = xT_cur[kc // 4][:, (kc % 4) * 128:(kc % 4 + 1) * 128]
                    nc.tensor.matmul(
                        qkv_ps[:, 0:512], lhsT=lhsT, rhs=w_sb[:, kc, 0:512],
                        start=(kc == 0), stop=(kc == KC - 1))
                    nc.tensor.matmul(
                        qkv_ps[:, 512:768], lhsT=lhsT, rhs=w_sb[:, kc, 512:768],
                        start=(kc == 0), stop=(kc == KC - 1))
                if t + 1 < KT:
                    xT_cur = x_transpose(t + 1, casts_next)

                # evict full qkv psum to SBUF fast (frees PSUM for next tile)
                qkv_sb = qsbp.tile([128, MQKV], f32, tag="qkv_sb")
                nc.scalar.copy(out=qkv_sb[:], in_=qkv_ps[:])
                # v eviction (no rope)
                nc.scalar.copy(out=v_sb[:, t, :], in_=qkv_sb[:, 640:768])

                # rope on q (4 heads) + k (1 head), pairs interleaved along free
                cs_t = csp.tile([128, 320], f32, tag="cs")
                nc.sync.dma_start(out=cs_t[:], in_=cs_ap[t * 128:(t + 1) * 128, :])
                sn_t = csp.tile([128, 320], f32, tag="sn")
                nc.sync.dma_start(out=sn_t[:], in_=sn_ap[t * 128:(t + 1) * 128, :])

                qk = qkv_sb[:, 0:640].rearrange("p (n two) -> p n two", two=2)
                qe = qk[:, :, 0]
                qo = qk[:, :, 1]
                rot = rotp.tile([128, 640], f16, tag="rot")
                rv = rot[:].rearrange("p (n two) -> p n two", two=2)
                t1 = rtp.tile([128, 320], f32, tag="t1")
                t2 = rtp.tile([128, 320], f32, tag="t2")
                nc.vector.tensor_mul(t1[:], qe, cs_t[:])
                nc.vector.tensor_mul(t2[:], qo, sn_t[:])
                nc.vector.scalar_tensor_tensor(
                    rv[:, :, 0], t2[:], -1.0, t1[:],
                    op0=mybir.AluOpType.mult, op1=mybir.AluOpType.add)
                nc.vector.tensor_mul(t1[:], qo, cs_t[:])
                nc.vector.tensor_mul(t2[:], qe, sn_t[:])
                nc.vector.tensor_add(rv[:, :, 1], t1[:], t2[:])

                # transpose rope'd q/k into [HD, head, t, 128] resident layout
                tr2 = ps_tr.tile([128, 512], f16, tag="tr")
                for h in range(4):
                    nc.tensor.transpose(
                        tr2[:, h * 128:(h + 1) * 128],
                        rot[:, h * 128:(h + 1) * 128], ident[:])
                nc.vector.tensor_copy(
                    out=qkT_sb[:, 0:4, t, :],
                    in_=tr2[:].rearrange("p (h s) -> p h s", h=4))
                tr3 = ps_tr.tile([128, 512], f16, tag="tr")
                nc.tensor.transpose(tr3[:, 0:128], rot[:, 512:640], ident[:])
                nc.vector.tensor_copy(out=qkT_sb[:, 4, t, :], in_=tr3[:, 0:128])

            # ---- phase 1 done: release w_sb space, load wo shard there
            w_pool_cm.__exit__(None, None, None)
            wo_pool = ctx.enter_context(tc.tile_pool(name="wo_pool", bufs=1, side="right"))
            wo_sb = wo_pool.tile([128, NQ, H], f16, name="wo_sb")
            wol = ctx.enter_context(tc.tile_pool(name="wol", bufs=2))
            for kc in range(NQ):
                for hh in range(4):
                    wt3 = wol.tile([128, 1024], f32, tag="wt3")
                    nc.sync.dma_start(
                        out=wt3[:],
                        in_=wo_ap[kc * 128:(kc + 1) * 128,
                                  hh * 1024:(hh + 1) * 1024])
                    nc.vector.tensor_copy(
                        out=wo_sb[:, kc, hh * 1024:(hh + 1) * 1024], in_=wt3[:])

            # ---- phase 2+3: causal flash attention with interleaved output
            # projection (y chunk q4 emitted once q-tiles 4*q4..4*q4+3 done)
            kT_flat = qkT_sb[:, 4, :, :].rearrange("p a b -> p (a b)")
            for i in range(KT):
                L = (i + 1) * 128
                nch = (L + 511) // 512
                Ps = []
                # wave 1: scores + exp + normalization chain for all heads
                for h in range(NQ):
                    P = pp.tile([128, S], f16, tag="P", name=f"P{i}_{h}")
                    lacc = lp.tile([128, 4], f32, tag="l")
                    for ch in range(nch):
                        c0 = ch * 512
                        c1 = min(L, c0 + 512)
                        sps = ps_big.tile([128, 512], f32, tag="big")
                        nc.tensor.matmul(
                            sps[:, 0:c1 - c0],
                            lhsT=qkT_sb[:, h, i, :],
                            rhs=kT_flat[:, c0:c1],
                            start=True, stop=True)
                        if c1 == L:
                            # diagonal block: additive causal mask
                            nc.vector.tensor_add(
                                sps[:, L - 128 - c0:L - c0],
                                sps[:, L - 128 - c0:L - c0], cmask[:])
                        nc.scalar.activation(
                            P[:, c0:c1], sps[:, 0:c1 - c0], Exp,
                            scale=SCALE,
                            accum_out=lacc[:, ch:ch + 1])
                    lsum = lp.tile([128, 1], f32, tag="ls")
                    if nch > 1:
                        nc.vector.tensor_reduce(
                            lsum[:], lacc[:, 0:nch],
                            axis=mybir.AxisListType.X, op=mybir.AluOpType.add)
                    else:
                        nc.vector.tensor_copy(out=lsum[:], in_=lacc[:, 0:1])
                    rinv = lp.tile([128, 1], f32, tag="r")
                    nc.vector.reciprocal(rinv[:], lsum[:])
                    nc.vector.tensor_scalar_mul(P[:, 0:L], P[:, 0:L], rinv[:])
                    Ps.append(P)
                # wave 2: transpose P and accumulate P^T @ v per head
                for h in range(NQ):
                    Pn = Ps[h]
                    PT = ptp.tile([128, S], f16, tag="PT", name=f"PT{i}_{h}")
                    for j4 in range(0, i + 1, 4):
                        jn = min(i + 1, j4 + 4)
                        trp = ps_tr.tile([128, 512], f16, tag="tr")
                        for jj in range(j4, jn):
                            nc.tensor.transpose(
                                trp[:, (jj - j4) * 128:(jj - j4 + 1) * 128],
                                Pn[:, jj * 128:(jj + 1) * 128], ident[:])
                        if (j4 // 4) % 2 == 0:
                            nc.scalar.copy(
                                out=PT[:, j4 * 128:jn * 128],
                                in_=trp[:, 0:(jn - j4) * 128])
                        else:
                            nc.vector.tensor_copy(
                                out=PT[:, j4 * 128:jn * 128],
                                in_=trp[:, 0:(jn - j4) * 128])

                    ov = ps_pv.tile([128, 128], f32, tag="pv")
                    for j in range(i + 1):
                        nc.tensor.matmul(
                            ov[:], lhsT=v_sb[:, j, :],
                            rhs=PT[:, j * 128:(j + 1) * 128],
                            start=(j == 0), stop=(j == i))
                    nc.vector.tensor_copy(
                        out=outT_sb[:, h, i * 128:(i + 1) * 128], in_=ov[:])

                yq4s = []
                if i % 4 == 3 and i >= 7:
                    yq4s = [i // 4 - 1]
                if i == KT - 1:
                    yq4s = [KT // 4 - 2, KT // 4 - 1]
                for q4 in yq4s:
                    for ym in range(H // 128):
                        yps = ps_big.tile([128, 512], f32, tag="big")
                        for kc in range(NQ):
                            nc.tensor.matmul(
                                yps[:],
                                lhsT=wo_sb[:, kc, ym * 128:(ym + 1) * 128],
                                rhs=outT_sb[:, kc, q4 * 512:(q4 + 1) * 512],
                                start=(kc == 0), stop=(kc == NQ - 1))
                        yev = yp.tile([128, 512], f32, tag="yev")
                        nc.vector.tensor_copy(out=yev[:], in_=yps[:])
                        nc.sync.dma_start(
                            out=yT_ap[ym * 128:(ym + 1) * 128,
                                      q4 * 512:(q4 + 1) * 512],
                            in_=yev[:])

    nc.compile()
    return nc


def _get_nc():
    if "nc" not in _CACHE:
        _CACHE["nc"] = _build()
    return _CACHE["nc"]


def kernel(x, last_pos, mask, rope_cache, wqkv, wo):
    global LAST_RESULTS
    from concourse.bass_utils import run_bass_kernel_spmd

    nc = _get_nc()

    x2 = np.ascontiguousarray(np.asarray(x, np.float32).reshape(S, H))
    rc = np.asarray(rope_cache, np.float32)          # [S, 64, 2]
    cos = rc[:, :, 0]                                # [S, 64]
    sin = rc[:, :, 1]
    # per-pair factors, tiled for 5 rope'd heads (4 q + 1 k): [S, 320]
    cs5 = np.ascontiguousarray(np.tile(cos, (1, 5)))
    sn5 = np.ascontiguousarray(np.tile(sin, (1, 5)))
    wq = np.asarray(wqkv, np.float32)
    wo_f = np.asarray(wo, np.float32)

    in_maps = []
    for c in range(N_CORES):
        wcat = np.concatenate(
            [wq[:, c * 512:(c + 1) * 512],
             wq[:, H + c * 128:H + (c + 1) * 128],
             wq[:, H + 1024 + c * 128:H + 1024 + (c + 1) * 128]],
            axis=1)
        in_maps.append({
            "x": x2,
            "w": np.ascontiguousarray(wcat),
            "wo": np.ascontiguousarray(wo_f[c * 512:(c + 1) * 512, :]),
            "cs5": cs5,
            "sn5": sn5,
        })

    res = run_bass_kernel_spmd(nc, in_maps, list(range(N_CORES)))
    LAST_RESULTS = res
    if res.exec_time_ns is not None:
        print(f"HW exec time: {res.exec_time_ns} ns")
    yT = res.results[0]["yT"].astype(np.float64)
    for c in range(1, N_CORES):
        yT = yT + res.results[c]["yT"]
    return np.ascontiguousarray(yT.T).reshape(1, S, H).astype(np.float32)
